# revision 30
# baseline (speedup 1.0000x reference)
"""Multi-head attention (B=8, N=1024, C=768, H=12) on 8 TRN2 NeuronCores.

Sharding: data-parallel over the batch - core i computes batch element i.
No collectives.

Per-core math (feature-major, no on-device transposes):
  qkT   = w_qkv[:, :1536].T @ xT            # [1536, 1024]
  v_tok = xT.T @ w_qkv[:, 1536:]            # [1024, 768] token-major + ones col
  per head h:
    ST   = k_h @ q_h^T                      # [1024k, 1024q]  K=64 matmuls; head
                                            #   pairs at PE tile positions
                                            #   (0,0)/(64,0), row-group-concurrent
    E    = exp(SCALE * ST)                  # bf16 on ACT, no max-subtraction
    [O_un; d] = [v_h | 1].T @ E             # [65, 512]: row 64 = softmax denom
    norm: rr = recip(d), rbc = gpsimd broadcast, ot = O_un * rr (DVE)
  yT = w_proj.T @ OT + b                    # bias on ACT post-exp, bf16 out

Schedule: the ACT exp stream (96 x [128,1024], ~104us serial) paces six
head-pair phases; the PE fills each phase's slack with v (phases 0-1), the
PV of pair p-2 (phases 2-5), and the next pairs' qk blocks, with nothing
heavy between a phase's last scores and the next phase's first. Score
tiles rotate three PSUM buffers (6 banks) so a kt's scores only WAR exps
from 1.5 kts back - keeps the hp0/hp1 row-group pairs issuing adjacently
(concurrent) instead of trailing the previous exp. PV/v/qk accumulation
chains share the remaining 2-bank pool; PV norms evict PSUM early (dd +
bf16 O copy at high priority) so the next chain never waits the full
recip/broadcast/mul tail. Head DMAs ride 3 queues (sync/gpsimd/scalar)
with per-k x tiles feeding a 4-chain qk prologue in arrival order, and
deferred loads (wv) strictly behind them in queue FIFO; wproj loads
mid-phase-2 on the idle sync queue. PE p-state warmup matmuls run under
the DMA head. The epilogue opens projection chunk pairs in freed score
slots (double-width [128,1024] opens), runs pair-5 PV/norm at high
priority with dd/O-copy and bias-adds moved to the post-exp-idle ACT
engine, closes each chunk with the k=5 block as norms land, and streams
bf16 outputs per chunk on the sync/scalar queues.

Host side: kernel() takes full inputs, pre-transposes/casts/packs
(partition-major lines), runs one SPMD NEFF on 8 cores, unpacks outputs.
HW exec ~177us at full clock (~188us baseline), rel err ~5.8e-3.
"""

import os
import sys

import numpy as np

for _p in ("/opt/trn_rl_repo", "/root/.axon_site/_ro/trn_rl_repo"):
    if os.path.isdir(_p) and _p not in sys.path:
        sys.path.insert(0, _p)

import concourse.bacc as bacc
import concourse.mybir as mybir
import concourse.tile as tile

F32 = mybir.dt.float32
BF16 = mybir.dt.bfloat16

B, NT, C = 8, 1024, 768
H, HD = 12, 64
C3 = 3 * C          # 2304
SCALE = HD ** -0.5  # 0.125
KT = C // 128       # 6   k-tiles over the C contraction
MQK = 1536 // 128   # 12  row-blocks of qkT
TT = NT // 128      # 8   token tiles
NQ = NT // 512      # 2   512-wide q slices
VA = HD + 1         # 65  v columns per head + ones column


def build_graph(tc):
    nc = tc.nc
    xt_d = nc.dram_tensor("xp", [128, KT * NT], BF16, kind="ExternalInput").ap()
    wqkp_d = nc.dram_tensor("wqkp", [6, 128, KT * 256], BF16, kind="ExternalInput").ap()
    wv_d = nc.dram_tensor("wvp", [128, KT * C], BF16, kind="ExternalInput").ap()
    wproj_d = nc.dram_tensor("wpp", [128, KT * C], BF16, kind="ExternalInput").ap()
    bp_d = nc.dram_tensor("bproj", [128, KT], F32, kind="ExternalInput").ap()
    out_d = nc.dram_tensor("out", [128, NQ * KT * 512], BF16, kind="ExternalOutput").ap()

    from contextlib import ExitStack

    with ExitStack() as stack:
        persist = stack.enter_context(tc.tile_pool(name="persist", bufs=1))
        qk_sb = persist.tile([128, MQK * NT], BF16)      # qkT feature-major
        vaug = persist.tile([128, TT * H * VA], BF16)    # [v_h | 1] per head, token-major
        ot03 = persist.tile([128, 4 * NT], BF16)         # attention out blocks 0-3
        ot4 = persist.tile([128, NT], BF16)              # block 4 (heads 8/9)
        ot5 = persist.tile([128, NT], BF16)              # block 5 (heads 10/11)

        def ot_ap(blk, p0, p1, c0, c1):
            if blk < 4:
                return ot03[p0:p1, blk * NT + c0 : blk * NT + c1]
            t = ot4 if blk == 4 else ot5
            return t[p0:p1, c0:c1]

        wv_sb = persist.tile([128, KT * C], BF16)          # v cols
        xt_sb = persist.tile([128, KT * NT], BF16)
        wp_sb = persist.tile([128, KT * C], BF16)
        bp_sb = persist.tile([128, KT], F32)
        ytp = persist.tile([128, NQ * KT * 512], BF16)     # bias-added proj out

        warm_in = persist.tile([1, 8], F32)
        warm_out = persist.tile([1, 8], BF16)
        wmm = persist.tile([1, 512], BF16)                 # PE warmup row

        attn = stack.enter_context(tc.tile_pool(name="attn", bufs=1))
        wqpool = stack.enter_context(tc.tile_pool(name="wqpool", bufs=2))
        ps_st = stack.enter_context(tc.tile_pool(name="ps_st", bufs=3, space="PSUM"))
        ps = stack.enter_context(tc.tile_pool(name="ps", bufs=2, space="PSUM"))

        wqp_t = {}

        def dma_qk_cols(b, eng=None):
            wqp_t[b] = wqpool.tile([128, KT * 256], BF16, name=f"wqp{b}", tag="wqp")
            (eng or nc.gpsimd).dma_start(out=wqp_t[b][:, :], in_=wqkp_d[b, :, :])

        # ---- constants / ACT exp-table preload (ASAP, ahead of queue use) ----
        nc.vector.memset(warm_in[:, :], 0.0)
        nc.scalar.activation(warm_out[:, :], warm_in[:, :],
                             mybir.ActivationFunctionType.Exp, scale=SCALE)
        nc.vector.memset(wmm[:, :], 0.0)
        nc.vector.memset(
            vaug[:, :].rearrange("p (g c) -> p g c", g=TT * H, c=VA)[:, :, HD : HD + 1],
            1.0,
        )

        # ---- critical DMAs: per-k-tile x lines balanced across the sync and
        #      gpsimd queues (engines round-robin across queues; FIFO within a
        #      queue keeps deferred loads from stealing head bandwidth) ----
        dma_qk_cols(0, nc.gpsimd)
        for k in (0, 2):
            nc.sync.dma_start(out=xt_sb[:, k * NT : (k + 1) * NT],
                              in_=xt_d[:, k * NT : (k + 1) * NT])
        for k in (1, 3):
            nc.gpsimd.dma_start(out=xt_sb[:, k * NT : (k + 1) * NT],
                                in_=xt_d[:, k * NT : (k + 1) * NT])
        for k in (4, 5):
            nc.scalar.dma_start(out=xt_sb[:, k * NT : (k + 1) * NT],
                                in_=xt_d[:, k * NT : (k + 1) * NT])
        # deferred loads ride behind the x tiles in queue-FIFO order
        dma_qk_cols(1, nc.sync)
        nc.gpsimd.dma_start(out=wv_sb[:, :], in_=wv_d[:, :])

        def dma_wproj():
            nc.sync.dma_start(out=wp_sb[:, :], in_=wproj_d[:, :])
            nc.sync.dma_start(out=bp_sb[:, :], in_=bp_d[:, :])

        # ---- PE p-state warmup while DMAs land ----
        wps = ps.tile([1, 512], F32, name="wps", tag="ps")
        for w in range(13):
            nc.tensor.matmul(wps[:, :], wmm[0:1, 0:1], wmm[0:1, :],
                             start=True, stop=True)

        v_ps = {}

        def emit_v_pair(t):
            emit_v_half(t, 0)
            emit_v_half(t, 1)

        def emit_v_half(t, half):
            """bank-interleaved 384-col chains; half 0 = k0-2, half 1 = k3-5"""
            if half == 0:
                v_ps[t] = (ps.tile([128, 384], F32, name=f"psv{t}_0", tag="ps"),
                           ps.tile([128, 384], F32, name=f"psv{t}_1", tag="ps"))
            pv0, pv1 = v_ps[t]
            for k in range(3 * half, 3 * half + 3):
                for j, psv in ((0, pv0), (1, pv1)):
                    nc.tensor.matmul(
                        psv[:, :],
                        xt_sb[:, k * NT + t * 128 : k * NT + (t + 1) * 128],
                        wv_sb[:, k * C + j * 384 : k * C + (j + 1) * 384],
                        start=(k == 0),
                        stop=(k == KT - 1),
                    )
            if half == 1:
                for j, psv in ((0, pv0), (1, pv1)):
                    h0 = 6 * j
                    nc.vector.tensor_copy(
                        vaug[:, t * H * VA + h0 * VA : t * H * VA + (h0 + 6) * VA]
                        .rearrange("p (g c) -> p g c", g=6, c=VA)[:, :, 0:HD],
                        psv[:, :].rearrange("p (g c) -> p g c", g=6, c=HD),
                    )

        # expst pair layout: pair p holds heads (2p, 2p+1);
        # slice for (h, kt, qs) = [:, kt*2048 + (h%2)*1024 + qs*512 :][:512]
        pair_tiles = {}

        cur_st = {}

        def emit_st_qs(p, kt, qs):
            """Scores for both heads of pair p, k-token-tile kt, q-slice qs.
            The hp0/hp1 matmuls land at PE tile positions (0,0)/(64,0) ->
            concurrent. Exps fire after the qs=1 half."""
            ep = pair_tiles[p]
            if qs == 0:
                cur_st[p] = [
                    ps_st.tile([128, 1024], F32, name=f"st{p}_{kt}_{hp}", tag="st")
                    for hp in range(2)
                ]
            tt = cur_st[p]
            # hp1 first: it carries the recent exp WAR, so the PE stalls once
            # and then both row-group matmuls issue adjacently (concurrent),
            # instead of hp0 running solo while hp1 waits
            for hp in (1, 0):
                p0 = hp * 64
                nc.tensor.matmul(
                    tt[hp][:, qs * 512 : qs * 512 + 512],
                    qk_sb[p0 : p0 + 64,
                          (6 + p) * NT + kt * 128 : (6 + p) * NT + (kt + 1) * 128],
                    qk_sb[p0 : p0 + 64,
                          p * NT + qs * 512 : p * NT + (qs + 1) * 512],
                    start=True,
                    stop=True,
                )
            if qs == 1:
                for hp in range(2):
                    nc.scalar.activation(
                        ep[:, kt * 2048 + hp * 1024 : kt * 2048 + hp * 1024 + 1024],
                        tt[hp][:, :],
                        mybir.ActivationFunctionType.Exp,
                        scale=SCALE,
                    )

        from contextlib import nullcontext

        def emit_norm(h, qs, pso, hi=False):
            """early psum evict (dd + bf16 O copy) at high priority; the
            recip/bcast/mul tail is off the PE-feeding path mid-phase but
            critical in the epilogue (closes wait on ot writes)."""
            p0 = (h % 2) * 64
            qblk = h // 2
            cpy = nc.scalar if hi else nc.vector
            with tc.high_priority():
                dd = attn.tile([1, 512], F32, name=f"dd{h}_{qs}", tag="dd", bufs=2)
                cpy.copy(dd[0:1, :], pso[64:65, :]) if hi else \
                    nc.vector.tensor_copy(dd[0:1, :], pso[64:65, :])
                oc = attn.tile([64, 512], BF16, name=f"oc{h}_{qs}", tag="oc", bufs=2)
                cpy.copy(oc[:, :], pso[0:64, :]) if hi else \
                    nc.vector.tensor_copy(oc[:, :], pso[0:64, :])
            with tc.high_priority() if hi else nullcontext():
                rr = attn.tile([1, 512], F32, name=f"rr{h}_{qs}", tag="rr", bufs=2)
                nc.vector.reciprocal_approx_fast(out=rr[0:1, :], in_=dd[0:1, :])
                rbc = attn.tile([64, 512], F32, name=f"rbc{h}_{qs}", tag="rbc", bufs=2)
                nc.gpsimd.partition_broadcast(rbc[:, :], rr[0:1, :])
                nc.vector.tensor_mul(
                    ot_ap(qblk, p0, p0 + 64, qs * 512, qs * 512 + 512),
                    oc[:, :],
                    rbc[:, :],
                )

        pv_ps = {}

        def emit_pv_half(h, qs, half, hi=False):
            """PV chain half (4 kts); norm fires after half 1 and overlaps
            subsequent PE work"""
            ep = pair_tiles[h // 2]
            if half == 0:
                pv_ps[(h, qs)] = ps.tile([VA, 512], F32,
                                         name=f"pso{h}_{qs}", tag="ps")
            pso = pv_ps[(h, qs)]
            for kt in range(4 * half, 4 * half + 4):
                nc.tensor.matmul(
                    pso[:, :],
                    vaug[:, kt * H * VA + h * VA : kt * H * VA + (h + 1) * VA],
                    ep[:, kt * 2048 + (h % 2) * 1024 + qs * 512 :
                       kt * 2048 + (h % 2) * 1024 + qs * 512 + 512],
                    start=(kt == 0),
                    stop=(kt == TT - 1),
                )
            if half == 1:
                emit_norm(h, qs, pso, hi=hi)
            return pso

        def emit_pv(h, qs, hi=False):
            emit_pv_half(h, qs, 0)
            return emit_pv_half(h, qs, 1, hi=hi)

        def emit_proj_open2(m0, ns):
            """open chunks (m0, m0+1) side by side in one score-pool slot"""
            psy = ps_st.tile([128, 1024], F32, name=f"psy{m0}_{ns}", tag="st")
            for k in range(5):
                for mi in range(2):
                    nc.tensor.matmul(
                        psy[:, mi * 512 : (mi + 1) * 512],
                        wp_sb[:, k * C + (m0 + mi) * 128 : k * C + (m0 + mi + 1) * 128],
                        ot_ap(k, 0, 128, ns * 512, (ns + 1) * 512),
                        start=(k == 0),
                        stop=False,
                    )
            return psy

        def emit_proj_close2(psy, m0, ns):
            k = 5
            for mi in range(2):
                m = m0 + mi
                nc.tensor.matmul(
                    psy[:, mi * 512 : (mi + 1) * 512],
                    wp_sb[:, k * C + m * 128 : k * C + (m + 1) * 128],
                    ot_ap(k, 0, 128, ns * 512, (ns + 1) * 512),
                    start=False,
                    stop=True,
                )
            for mi in range(2):
                m = m0 + mi
                yslice = ytp[:, ns * KT * 512 + m * 512 : ns * KT * 512 + (m + 1) * 512]
                if mi == 0:
                    nc.scalar.add(
                        yslice, psy[:, mi * 512 : (mi + 1) * 512], bp_sb[:, m : m + 1])
                else:
                    nc.vector.tensor_scalar_add(
                        yslice, psy[:, mi * 512 : (mi + 1) * 512], bp_sb[:, m : m + 1])
                eng = nc.scalar if m % 2 else nc.sync
                eng.dma_start(
                    out=out_d[:, ns * KT * 512 + m * 512 : ns * KT * 512 + (m + 1) * 512],
                    in_=yslice,
                )

        def emit_qk_group(m, n):
            bb, half = m % 6, (0 if m < 6 else 1)
            psq = ps.tile([128, 512], F32, name=f"psq{m}_{n}", tag="ps")
            for k in range(KT):
                nc.tensor.matmul(
                    psq[:, :],
                    wqp_t[bb][:, k * 256 + half * 128 : k * 256 + half * 128 + 128],
                    xt_sb[:, k * NT + n * 512 : k * NT + (n + 1) * 512],
                    start=(k == 0),
                    stop=(k == KT - 1),
                )
            nc.vector.tensor_copy(
                qk_sb[:, m * NT + n * 512 : m * NT + n * 512 + 512], psq[:, :]
            )

        # ---- prologue: all four pair-0 qk chains ride the k-tile DMA
        #      arrivals (2 chains on ps banks, 2 on score banks) ----
        quad = []
        for n in range(NQ):
            for m in (6, 0):
                bb, half = m % 6, (0 if m < 6 else 1)
                pl, tg = (ps, "ps") if n == 0 else (ps_st, "st")
                psq = pl.tile([128, 512], F32, name=f"psq{m}_{n}", tag=tg)
                quad.append((m, half, n, psq))
        korder = (0, 4, 2, 5, 1, 3)
        for i, k in enumerate(korder):
            for m, half, n, psq in quad:
                nc.tensor.matmul(
                    psq[:, :],
                    wqp_t[0][:, k * 256 + half * 128 : k * 256 + half * 128 + 128],
                    xt_sb[:, k * NT + n * 512 : k * NT + (n + 1) * 512],
                    start=(i == 0),
                    stop=(i == KT - 1),
                )
        for m, half, n, psq in quad:
            nc.vector.tensor_copy(
                qk_sb[:, m * NT + n * 512 : m * NT + n * 512 + 512], psq[:, :]
            )

        # ---- main loop over head pairs; per-phase fillers sized to the
        #      exp window, nothing heavy after the kt6 slot ----
        held = {}
        phase_fillers = {
            0: [
                lambda: emit_qk_group(1, 0),
                lambda: emit_qk_group(1, 1),
                lambda: emit_v_pair(0),
                lambda: emit_v_pair(1),
                lambda: emit_v_pair(2),
                lambda: emit_v_pair(3),
                lambda: (emit_qk_group(7, 0), emit_qk_group(7, 1)),
            ],
            1: [
                lambda: (dma_qk_cols(2, nc.sync), emit_v_pair(4)),
                lambda: emit_v_pair(5),
                lambda: emit_v_pair(6),
                lambda: emit_v_pair(7),
                lambda: (emit_qk_group(2, 0), emit_qk_group(2, 1)),
                lambda: (emit_qk_group(8, 0), emit_qk_group(8, 1)),
            ],
        }
        for b in (2, 3, 4):
            phase_fillers[b] = [
                lambda b=b: (dma_qk_cols(b + 1, nc.sync),
                             dma_wproj() if b == 2 else None,
                             emit_pv(2 * b - 4, 0)),
                lambda b=b: emit_pv(2 * b - 3, 0),
                lambda b=b: emit_qk_group(b + 1, 0),
                lambda b=b: emit_pv(2 * b - 4, 1),
                lambda b=b: emit_pv(2 * b - 3, 1),
                lambda b=b: emit_qk_group(b + 1, 1),
                lambda b=b: (emit_qk_group(7 + b, 0), emit_qk_group(7 + b, 1)),
            ]
        phase_fillers[5] = [
            lambda: emit_pv(6, 0),
            lambda: emit_pv(7, 0),
            lambda: emit_pv(6, 1),
            lambda: emit_pv(7, 1),
            lambda: emit_pv(8, 0),
            lambda: emit_pv(9, 0),
            lambda: emit_pv(8, 1),
        ]
        for b in range(6):
            fillers = phase_fillers[b]
            pair_tiles[b] = attn.tile([128, TT * 2048], BF16, name=f"epair{b}",
                                      tag="epair", bufs=3)
            fi = 0
            for kt in range(TT):
                with tc.high_priority():
                    emit_st_qs(b, kt, 0)
                    emit_st_qs(b, kt, 1)
                if kt < TT - 1 and fi < len(fillers):
                    fillers[fi]()
                    fi += 1
            while fi < len(fillers):
                fillers[fi]()
                fi += 1

        # ---- epilogue: remaining pair-4 PV, double-width projection opens
        #      on freed score slots, pair-5 PV/norm on the shared ps banks,
        #      closes as norms land; bf16 output DMAs on sync/scalar ----
        emit_pv(9, 1)
        d1 = emit_proj_open2(0, 0)
        d2 = emit_proj_open2(2, 0)
        d3 = emit_proj_open2(0, 1)
        emit_pv(10, 0, hi=True)
        emit_pv(11, 0, hi=True)
        emit_pv(10, 1, hi=True)
        emit_pv(11, 1, hi=True)
        emit_proj_close2(d1, 0, 0)
        emit_proj_close2(d2, 2, 0)
        d4 = emit_proj_open2(2, 1)
        d5 = emit_proj_open2(4, 0)
        emit_proj_close2(d3, 0, 1)
        d6 = emit_proj_open2(4, 1)
        emit_proj_close2(d4, 2, 1)
        emit_proj_close2(d5, 4, 0)
        emit_proj_close2(d6, 4, 1)


_NC = None


def build_nc():
    global _NC
    if _NC is None:
        nc = bacc.Bacc(
            trn_type="TRN2",
            target_bir_lowering=False,
            debug=False,
            enable_asserts=False,
            num_devices=8,
        )
        with tile.TileContext(nc) as tc:
            build_graph(tc)
        nc.compile()
        _NC = nc
    return _NC


def make_in_maps(x, w_qkv, w_proj, b_proj):
    import ml_dtypes

    bf16 = ml_dtypes.bfloat16
    x = np.asarray(x, dtype=np.float32)
    w_qkv = np.asarray(w_qkv, dtype=np.float32).astype(bf16)
    w_proj = np.asarray(w_proj, dtype=np.float32).astype(bf16)
    b_proj = np.asarray(b_proj, dtype=np.float32)
    xT = np.ascontiguousarray(x.transpose(0, 2, 1).astype(bf16))  # [8, 768, 1024]
    # partition-major packs: row p holds k-tile slices back to back
    xp = np.ascontiguousarray(
        xT.reshape(B, KT, 128, NT).transpose(0, 2, 1, 3).reshape(B, 128, KT * NT))
    wvp = np.ascontiguousarray(
        w_qkv[:, 1536:].reshape(KT, 128, C).transpose(1, 0, 2).reshape(128, KT * C))
    wpp = np.ascontiguousarray(
        w_proj.reshape(KT, 128, C).transpose(1, 0, 2).reshape(128, KT * C))
    # per-pair packed q/k col blocks in SBUF layout: [6, 128, KT*256]
    wqkp = np.empty((6, 128, KT * 256), dtype=bf16)
    for b in range(6):
        blk = np.concatenate(
            [w_qkv[:, b * 128 : (b + 1) * 128],
             w_qkv[:, 768 + b * 128 : 768 + (b + 1) * 128]], axis=1)  # [768, 256]
        wqkp[b] = np.ascontiguousarray(
            blk.reshape(KT, 128, 256).transpose(1, 0, 2).reshape(128, KT * 256))
    bp = np.ascontiguousarray(b_proj.reshape(KT, 128).T)          # [128, 6]
    return [
        {"xp": xp[i], "wqkp": wqkp, "wvp": wvp, "wpp": wpp, "bproj": bp}
        for i in range(B)
    ]


def run_on_hw(in_maps, trace=False, **kwargs):
    from concourse.bass_utils import run_bass_kernel_spmd

    nc = build_nc()
    return run_bass_kernel_spmd(
        nc, in_maps, core_ids=list(range(B)), trace=trace, **kwargs
    )


def _unpack_out(o):
    # o: [128, 2*6*512] bf16, col = ns*3072 + m*512 -> y [768, 1024] -> [1024, 768]
    y = np.asarray(o).reshape(128, NQ, KT, 512).transpose(2, 0, 1, 3).reshape(C, NT)
    return y.T.astype(np.float32)


def kernel(x, w_qkv, w_proj, b_proj):
    in_maps = make_in_maps(x, w_qkv, w_proj, b_proj)
    res = run_on_hw(in_maps, trace=False)
    out = np.stack([_unpack_out(res.results[i]["out"]) for i in range(B)])
    return np.ascontiguousarray(out)


# revision 31
# speedup vs baseline: 1.0894x; 1.0894x over previous
"""Multi-head attention (B=8, N=1024, C=768, H=12) on 8 TRN2 NeuronCores.

Sharding: data-parallel over the batch - core i computes batch element i.
No collectives.

Per-core math (feature-major, no on-device transposes):
  qkT   = w_qkv[:, :1536].T @ xT            # [1536, 1024]
  v_tok = xT.T @ w_qkv[:, 1536:]            # [1024, 768] token-major + ones col
  per head h:
    ST   = k_h @ q_h^T                      # [1024k, 1024q]  K=64 matmuls; head
                                            #   pairs at PE tile positions
                                            #   (0,0)/(64,0), row-group-concurrent
    E    = exp(SCALE * ST)                  # bf16 on ACT, no max-subtraction
    [O_un; d] = [v_h | 1].T @ E             # [65, 512]: row 64 = softmax denom
    norm: rr = recip(d), rbc = gpsimd broadcast, ot = O_un * rr (DVE)
  yT = w_proj.T @ OT + b                    # bias on ACT post-exp, bf16 out

Schedule: the ACT exp stream (96 x [128,1024], ~104us serial) paces six
head-pair phases; the PE fills each phase's slack with v (phases 0-1), the
PV of pair p-2 (phases 2-5), and the next pairs' qk blocks, with nothing
heavy between a phase's last scores and the next phase's first. Score
tiles rotate three PSUM buffers (6 banks) so a kt's scores only WAR exps
from 1.5 kts back - keeps the hp0/hp1 row-group pairs issuing adjacently
(concurrent) instead of trailing the previous exp. PV/v/qk accumulation
chains share the remaining 2-bank pool; PV norms evict PSUM early (dd +
bf16 O copy at high priority) so the next chain never waits the full
recip/broadcast/mul tail. Head DMAs ride 3 queues (sync/gpsimd/scalar)
with per-k x tiles feeding a 4-chain qk prologue in arrival order, and
deferred loads (wv) strictly behind them in queue FIFO; wproj loads
mid-phase-2 on the idle sync queue. PE p-state warmup matmuls run under
the DMA head. The epilogue opens projection chunk pairs in freed score
slots (double-width [128,1024] opens), runs pair-5 PV/norm at high
priority with dd/O-copy and bias-adds moved to the post-exp-idle ACT
engine, closes each chunk with the k=5 block as norms land, and streams
bf16 outputs per chunk on the sync/scalar queues.

Host side: kernel() takes full inputs, pre-transposes/casts/packs
(partition-major lines), runs one SPMD NEFF on 8 cores, unpacks outputs.
HW exec ~177us at full clock (~188us baseline), rel err ~5.8e-3.
"""

import os
import sys

import numpy as np

for _p in ("/opt/trn_rl_repo", "/root/.axon_site/_ro/trn_rl_repo"):
    if os.path.isdir(_p) and _p not in sys.path:
        sys.path.insert(0, _p)

import concourse.bacc as bacc
import concourse.mybir as mybir
import concourse.tile as tile

F32 = mybir.dt.float32
BF16 = mybir.dt.bfloat16

B, NT, C = 8, 1024, 768
H, HD = 12, 64
C3 = 3 * C          # 2304
SCALE = HD ** -0.5  # 0.125
KT = C // 128       # 6   k-tiles over the C contraction
MQK = 1536 // 128   # 12  row-blocks of qkT
TT = NT // 128      # 8   token tiles
NQ = NT // 512      # 2   512-wide q slices
VA = HD + 1         # 65  v columns per head + ones column


def build_graph(tc):
    nc = tc.nc
    xt_d = nc.dram_tensor("xp", [128, KT * NT], BF16, kind="ExternalInput").ap()
    wqkp_d = nc.dram_tensor("wqkp", [6, 128, KT * 256], BF16, kind="ExternalInput").ap()
    wv_d = nc.dram_tensor("wvp", [128, KT * C], BF16, kind="ExternalInput").ap()
    wproj_d = nc.dram_tensor("wpp", [128, KT * C], BF16, kind="ExternalInput").ap()
    bp_d = nc.dram_tensor("bproj", [128, KT], F32, kind="ExternalInput").ap()
    out_d = nc.dram_tensor("out", [128, NQ * KT * 512], BF16, kind="ExternalOutput").ap()

    from contextlib import ExitStack

    with ExitStack() as stack:
        persist = stack.enter_context(tc.tile_pool(name="persist", bufs=1))
        qk_sb = persist.tile([128, MQK * NT], BF16)      # qkT feature-major
        vaug = persist.tile([128, TT * H * VA], BF16)    # [v_h | 1] per head, token-major
        ot03 = persist.tile([128, 4 * NT], BF16)         # attention out blocks 0-3
        ot4 = persist.tile([128, NT], BF16)              # block 4 (heads 8/9)
        ot5 = persist.tile([128, NT], BF16)              # block 5 (heads 10/11)

        def ot_ap(blk, p0, p1, c0, c1):
            if blk < 4:
                return ot03[p0:p1, blk * NT + c0 : blk * NT + c1]
            t = ot4 if blk == 4 else ot5
            return t[p0:p1, c0:c1]

        wv_sb = persist.tile([128, KT * C], BF16)          # v cols
        xt_sb = persist.tile([128, KT * NT], BF16)
        wp_sb = persist.tile([128, KT * C], BF16)
        bp_sb = persist.tile([128, KT], F32)
        ytp = persist.tile([128, NQ * KT * 512], BF16)     # bias-added proj out

        warm_in = persist.tile([1, 8], F32)
        warm_out = persist.tile([1, 8], BF16)
        wmm = persist.tile([1, 512], BF16)                 # PE warmup row

        attn = stack.enter_context(tc.tile_pool(name="attn", bufs=1))
        wqpool = stack.enter_context(tc.tile_pool(name="wqpool", bufs=2))
        ps_st = stack.enter_context(tc.tile_pool(name="ps_st", bufs=3, space="PSUM"))
        ps = stack.enter_context(tc.tile_pool(name="ps", bufs=2, space="PSUM"))

        wqp_t = {}

        def dma_qk_cols(b, eng=None):
            wqp_t[b] = wqpool.tile([128, KT * 256], BF16, name=f"wqp{b}", tag="wqp")
            (eng or nc.gpsimd).dma_start(out=wqp_t[b][:, :], in_=wqkp_d[b, :, :])

        # ---- constants / ACT exp-table preload (ASAP, ahead of queue use) ----
        nc.vector.memset(warm_in[:, :], 0.0)
        nc.scalar.activation(warm_out[:, :], warm_in[:, :],
                             mybir.ActivationFunctionType.Exp, scale=SCALE)
        nc.vector.memset(wmm[:, :], 0.0)
        nc.vector.memset(
            vaug[:, :].rearrange("p (g c) -> p g c", g=TT * H, c=VA)[:, :, HD : HD + 1],
            1.0,
        )

        # ---- critical DMAs: per-k-tile x lines balanced across the sync and
        #      gpsimd queues (engines round-robin across queues; FIFO within a
        #      queue keeps deferred loads from stealing head bandwidth) ----
        dma_qk_cols(0, nc.gpsimd)
        for k in (0, 2):
            nc.sync.dma_start(out=xt_sb[:, k * NT : (k + 1) * NT],
                              in_=xt_d[:, k * NT : (k + 1) * NT])
        for k in (1, 3):
            nc.gpsimd.dma_start(out=xt_sb[:, k * NT : (k + 1) * NT],
                                in_=xt_d[:, k * NT : (k + 1) * NT])
        for k in (4, 5):
            nc.scalar.dma_start(out=xt_sb[:, k * NT : (k + 1) * NT],
                                in_=xt_d[:, k * NT : (k + 1) * NT])
        # deferred loads ride behind the x tiles in queue-FIFO order
        dma_qk_cols(1, nc.sync)
        nc.gpsimd.dma_start(out=wv_sb[:, :], in_=wv_d[:, :])

        def dma_wproj():
            nc.sync.dma_start(out=wp_sb[:, :], in_=wproj_d[:, :])
            nc.sync.dma_start(out=bp_sb[:, :], in_=bp_d[:, :])

        # ---- PE p-state warmup while DMAs land ----
        wps = ps.tile([1, 512], F32, name="wps", tag="ps")
        for w in range(13):
            nc.tensor.matmul(wps[:, :], wmm[0:1, 0:1], wmm[0:1, :],
                             start=True, stop=True)

        v_ps = {}

        def emit_v_pair(t):
            emit_v_half(t, 0)
            emit_v_half(t, 1)

        def emit_v_half(t, half):
            """bank-interleaved 384-col chains; half 0 = k0-2, half 1 = k3-5"""
            if half == 0:
                v_ps[t] = (ps.tile([128, 384], F32, name=f"psv{t}_0", tag="ps"),
                           ps.tile([128, 384], F32, name=f"psv{t}_1", tag="ps"))
            pv0, pv1 = v_ps[t]
            for k in range(3 * half, 3 * half + 3):
                for j, psv in ((0, pv0), (1, pv1)):
                    nc.tensor.matmul(
                        psv[:, :],
                        xt_sb[:, k * NT + t * 128 : k * NT + (t + 1) * 128],
                        wv_sb[:, k * C + j * 384 : k * C + (j + 1) * 384],
                        start=(k == 0),
                        stop=(k == KT - 1),
                    )
            if half == 1:
                for j, psv in ((0, pv0), (1, pv1)):
                    h0 = 6 * j
                    nc.vector.tensor_copy(
                        vaug[:, t * H * VA + h0 * VA : t * H * VA + (h0 + 6) * VA]
                        .rearrange("p (g c) -> p g c", g=6, c=VA)[:, :, 0:HD],
                        psv[:, :].rearrange("p (g c) -> p g c", g=6, c=HD),
                    )

        # expst pair layout: pair p holds heads (2p, 2p+1);
        # slice for (h, kt, qs) = [:, kt*2048 + (h%2)*1024 + qs*512 :][:512]
        pair_tiles = {}

        cur_st = {}

        def emit_st_qs(p, kt, qs):
            """Scores for both heads of pair p, k-token-tile kt, q-slice qs.
            The hp0/hp1 matmuls land at PE tile positions (0,0)/(64,0) ->
            concurrent. Exps fire after the qs=1 half."""
            ep = pair_tiles[p]
            if qs == 0:
                cur_st[p] = [
                    ps_st.tile([128, 1024], F32, name=f"st{p}_{kt}_{hp}", tag="st")
                    for hp in range(2)
                ]
            tt = cur_st[p]
            # hp1 first: it carries the recent exp WAR, so the PE stalls once
            # and then both row-group matmuls issue adjacently (concurrent),
            # instead of hp0 running solo while hp1 waits
            for hp in (1, 0):
                p0 = hp * 64
                nc.tensor.matmul(
                    tt[hp][:, qs * 512 : qs * 512 + 512],
                    qk_sb[p0 : p0 + 64,
                          (6 + p) * NT + kt * 128 : (6 + p) * NT + (kt + 1) * 128],
                    qk_sb[p0 : p0 + 64,
                          p * NT + qs * 512 : p * NT + (qs + 1) * 512],
                    start=True,
                    stop=True,
                )
            if qs == 1:
                for hp in range(2):
                    nc.scalar.activation(
                        ep[:, kt * 2048 + hp * 1024 : kt * 2048 + hp * 1024 + 1024],
                        tt[hp][:, :],
                        mybir.ActivationFunctionType.Exp,
                        scale=SCALE,
                    )

        from contextlib import nullcontext

        def emit_norm(h, qs, pso, hi=False):
            """early psum evict (dd + bf16 O copy) at high priority; the
            recip/bcast/mul tail is off the PE-feeding path mid-phase but
            critical in the epilogue (closes wait on ot writes)."""
            p0 = (h % 2) * 64
            qblk = h // 2
            cpy = nc.scalar if hi else nc.vector
            with tc.high_priority():
                dd = attn.tile([1, 512], F32, name=f"dd{h}_{qs}", tag="dd", bufs=2)
                cpy.copy(dd[0:1, :], pso[64:65, :]) if hi else \
                    nc.vector.tensor_copy(dd[0:1, :], pso[64:65, :])
                oc = attn.tile([64, 512], BF16, name=f"oc{h}_{qs}", tag="oc", bufs=2)
                cpy.copy(oc[:, :], pso[0:64, :]) if hi else \
                    nc.vector.tensor_copy(oc[:, :], pso[0:64, :])
            with tc.high_priority() if hi else nullcontext():
                rr = attn.tile([1, 512], F32, name=f"rr{h}_{qs}", tag="rr", bufs=2)
                nc.vector.reciprocal_approx_fast(out=rr[0:1, :], in_=dd[0:1, :])
                rbc = attn.tile([64, 512], F32, name=f"rbc{h}_{qs}", tag="rbc", bufs=2)
                nc.gpsimd.partition_broadcast(rbc[:, :], rr[0:1, :])
                nc.vector.tensor_mul(
                    ot_ap(qblk, p0, p0 + 64, qs * 512, qs * 512 + 512),
                    oc[:, :],
                    rbc[:, :],
                )

        pv_ps = {}

        def emit_pv_half(h, qs, half, hi=False):
            """PV chain half (4 kts); norm fires after half 1 and overlaps
            subsequent PE work"""
            ep = pair_tiles[h // 2]
            if half == 0:
                pv_ps[(h, qs)] = ps.tile([VA, 512], F32,
                                         name=f"pso{h}_{qs}", tag="ps")
            pso = pv_ps[(h, qs)]
            for kt in range(4 * half, 4 * half + 4):
                nc.tensor.matmul(
                    pso[:, :],
                    vaug[:, kt * H * VA + h * VA : kt * H * VA + (h + 1) * VA],
                    ep[:, kt * 2048 + (h % 2) * 1024 + qs * 512 :
                       kt * 2048 + (h % 2) * 1024 + qs * 512 + 512],
                    start=(kt == 0),
                    stop=(kt == TT - 1),
                )
            if half == 1:
                emit_norm(h, qs, pso, hi=hi)
            return pso

        def emit_pv(h, qs, hi=False):
            emit_pv_half(h, qs, 0)
            return emit_pv_half(h, qs, 1, hi=hi)

        def emit_proj_open2(m0, ns):
            """open chunks (m0, m0+1) side by side in one score-pool slot"""
            psy = ps_st.tile([128, 1024], F32, name=f"psy{m0}_{ns}", tag="st")
            for k in range(5):
                for mi in range(2):
                    nc.tensor.matmul(
                        psy[:, mi * 512 : (mi + 1) * 512],
                        wp_sb[:, k * C + (m0 + mi) * 128 : k * C + (m0 + mi + 1) * 128],
                        ot_ap(k, 0, 128, ns * 512, (ns + 1) * 512),
                        start=(k == 0),
                        stop=False,
                    )
            return psy

        def emit_proj_close2(psy, m0, ns):
            k = 5
            for mi in range(2):
                m = m0 + mi
                nc.tensor.matmul(
                    psy[:, mi * 512 : (mi + 1) * 512],
                    wp_sb[:, k * C + m * 128 : k * C + (m + 1) * 128],
                    ot_ap(k, 0, 128, ns * 512, (ns + 1) * 512),
                    start=False,
                    stop=True,
                )
            for mi in range(2):
                m = m0 + mi
                yslice = ytp[:, ns * KT * 512 + m * 512 : ns * KT * 512 + (m + 1) * 512]
                if mi == 0:
                    nc.scalar.add(
                        yslice, psy[:, mi * 512 : (mi + 1) * 512], bp_sb[:, m : m + 1])
                else:
                    nc.vector.tensor_scalar_add(
                        yslice, psy[:, mi * 512 : (mi + 1) * 512], bp_sb[:, m : m + 1])
                eng = nc.scalar if m % 2 else nc.sync
                eng.dma_start(
                    out=out_d[:, ns * KT * 512 + m * 512 : ns * KT * 512 + (m + 1) * 512],
                    in_=yslice,
                )

        def emit_qk_group(m, n):
            bb, half = m % 6, (0 if m < 6 else 1)
            psq = ps.tile([128, 512], F32, name=f"psq{m}_{n}", tag="ps")
            for k in range(KT):
                nc.tensor.matmul(
                    psq[:, :],
                    wqp_t[bb][:, k * 256 + half * 128 : k * 256 + half * 128 + 128],
                    xt_sb[:, k * NT + n * 512 : k * NT + (n + 1) * 512],
                    start=(k == 0),
                    stop=(k == KT - 1),
                )
            nc.vector.tensor_copy(
                qk_sb[:, m * NT + n * 512 : m * NT + n * 512 + 512], psq[:, :]
            )

        # ---- prologue: all four pair-0 qk chains ride the k-tile DMA
        #      arrivals (2 chains on ps banks, 2 on score banks) ----
        quad = []
        for n in range(NQ):
            for m in (6, 0):
                bb, half = m % 6, (0 if m < 6 else 1)
                pl, tg = (ps, "ps") if n == 0 else (ps_st, "st")
                psq = pl.tile([128, 512], F32, name=f"psq{m}_{n}", tag=tg)
                quad.append((m, half, n, psq))
        korder = (0, 4, 2, 5, 1, 3)
        for i, k in enumerate(korder):
            for m, half, n, psq in quad:
                nc.tensor.matmul(
                    psq[:, :],
                    wqp_t[0][:, k * 256 + half * 128 : k * 256 + half * 128 + 128],
                    xt_sb[:, k * NT + n * 512 : k * NT + (n + 1) * 512],
                    start=(i == 0),
                    stop=(i == KT - 1),
                )
        for m, half, n, psq in quad:
            nc.vector.tensor_copy(
                qk_sb[:, m * NT + n * 512 : m * NT + n * 512 + 512], psq[:, :]
            )

        # ---- main loop over head pairs; per-phase fillers sized to the
        #      exp window, nothing heavy after the kt6 slot ----
        held = {}
        phase_fillers = {
            0: [
                lambda: emit_qk_group(1, 0),
                lambda: emit_qk_group(1, 1),
                lambda: emit_v_pair(0),
                lambda: emit_v_pair(1),
                lambda: emit_v_pair(2),
                lambda: emit_v_pair(3),
                lambda: (emit_qk_group(7, 0), emit_qk_group(7, 1)),
            ],
            1: [
                lambda: (dma_qk_cols(2, nc.sync), emit_v_pair(4)),
                lambda: emit_v_pair(5),
                lambda: emit_v_pair(6),
                lambda: emit_v_pair(7),
                lambda: (emit_qk_group(2, 0), emit_qk_group(2, 1)),
                lambda: (emit_qk_group(8, 0), emit_qk_group(8, 1)),
            ],
        }
        for b in (2, 3, 4):
            phase_fillers[b] = [
                lambda b=b: (dma_qk_cols(b + 1, nc.sync),
                             dma_wproj() if b == 2 else None,
                             emit_pv(2 * b - 4, 0)),
                lambda b=b: emit_pv(2 * b - 3, 0),
                lambda b=b: emit_qk_group(b + 1, 0),
                lambda b=b: emit_pv(2 * b - 4, 1),
                lambda b=b: emit_pv(2 * b - 3, 1),
                lambda b=b: emit_qk_group(b + 1, 1),
                lambda b=b: (emit_qk_group(7 + b, 0), emit_qk_group(7 + b, 1)),
            ]
        phase_fillers[5] = [
            lambda: emit_pv(6, 0),
            lambda: emit_pv(7, 0),
            lambda: emit_pv(6, 1),
            lambda: emit_pv(7, 1),
            lambda: emit_pv(8, 0),
            lambda: emit_pv(9, 0),
            lambda: emit_pv(8, 1),
        ]
        for b in range(6):
            fillers = phase_fillers[b]
            pair_tiles[b] = attn.tile([128, TT * 2048], BF16, name=f"epair{b}",
                                      tag="epair", bufs=3)
            fi = 0
            for kt in range(TT):
                emit_st_qs(b, kt, 0)
                emit_st_qs(b, kt, 1)
                if kt < TT - 1 and fi < len(fillers):
                    fillers[fi]()
                    fi += 1
            while fi < len(fillers):
                fillers[fi]()
                fi += 1

        # ---- epilogue: remaining pair-4 PV, double-width projection opens
        #      on freed score slots, pair-5 PV/norm on the shared ps banks,
        #      closes as norms land; bf16 output DMAs on sync/scalar ----
        emit_pv(9, 1)
        d1 = emit_proj_open2(0, 0)
        d2 = emit_proj_open2(2, 0)
        d3 = emit_proj_open2(0, 1)
        emit_pv(10, 0, hi=True)
        emit_pv(11, 0, hi=True)
        emit_pv(10, 1, hi=True)
        emit_pv(11, 1, hi=True)
        emit_proj_close2(d1, 0, 0)
        emit_proj_close2(d2, 2, 0)
        d4 = emit_proj_open2(2, 1)
        d5 = emit_proj_open2(4, 0)
        emit_proj_close2(d3, 0, 1)
        d6 = emit_proj_open2(4, 1)
        emit_proj_close2(d4, 2, 1)
        emit_proj_close2(d5, 4, 0)
        emit_proj_close2(d6, 4, 1)


_NC = None


def build_nc():
    global _NC
    if _NC is None:
        nc = bacc.Bacc(
            trn_type="TRN2",
            target_bir_lowering=False,
            debug=False,
            enable_asserts=False,
            num_devices=8,
        )
        with tile.TileContext(nc) as tc:
            build_graph(tc)
        nc.compile()
        _NC = nc
    return _NC


def make_in_maps(x, w_qkv, w_proj, b_proj):
    import ml_dtypes

    bf16 = ml_dtypes.bfloat16
    x = np.asarray(x, dtype=np.float32)
    w_qkv = np.asarray(w_qkv, dtype=np.float32).astype(bf16)
    w_proj = np.asarray(w_proj, dtype=np.float32).astype(bf16)
    b_proj = np.asarray(b_proj, dtype=np.float32)
    xT = np.ascontiguousarray(x.transpose(0, 2, 1).astype(bf16))  # [8, 768, 1024]
    # partition-major packs: row p holds k-tile slices back to back
    xp = np.ascontiguousarray(
        xT.reshape(B, KT, 128, NT).transpose(0, 2, 1, 3).reshape(B, 128, KT * NT))
    wvp = np.ascontiguousarray(
        w_qkv[:, 1536:].reshape(KT, 128, C).transpose(1, 0, 2).reshape(128, KT * C))
    wpp = np.ascontiguousarray(
        w_proj.reshape(KT, 128, C).transpose(1, 0, 2).reshape(128, KT * C))
    # per-pair packed q/k col blocks in SBUF layout: [6, 128, KT*256]
    wqkp = np.empty((6, 128, KT * 256), dtype=bf16)
    for b in range(6):
        blk = np.concatenate(
            [w_qkv[:, b * 128 : (b + 1) * 128],
             w_qkv[:, 768 + b * 128 : 768 + (b + 1) * 128]], axis=1)  # [768, 256]
        wqkp[b] = np.ascontiguousarray(
            blk.reshape(KT, 128, 256).transpose(1, 0, 2).reshape(128, KT * 256))
    bp = np.ascontiguousarray(b_proj.reshape(KT, 128).T)          # [128, 6]
    return [
        {"xp": xp[i], "wqkp": wqkp, "wvp": wvp, "wpp": wpp, "bproj": bp}
        for i in range(B)
    ]


def run_on_hw(in_maps, trace=False, **kwargs):
    from concourse.bass_utils import run_bass_kernel_spmd

    nc = build_nc()
    return run_bass_kernel_spmd(
        nc, in_maps, core_ids=list(range(B)), trace=trace, **kwargs
    )


def _unpack_out(o):
    # o: [128, 2*6*512] bf16, col = ns*3072 + m*512 -> y [768, 1024] -> [1024, 768]
    y = np.asarray(o).reshape(128, NQ, KT, 512).transpose(2, 0, 1, 3).reshape(C, NT)
    return y.T.astype(np.float32)


def kernel(x, w_qkv, w_proj, b_proj):
    in_maps = make_in_maps(x, w_qkv, w_proj, b_proj)
    res = run_on_hw(in_maps, trace=False)
    out = np.stack([_unpack_out(res.results[i]["out"]) for i in range(B)])
    return np.ascontiguousarray(out)
